# revision 1
# baseline (speedup 1.0000x reference)
"""TRN2 Bass kernel: masked multi-head attention block (B=4, S=2048, C=768, H=12).

Sharding: 8 cores = 4 batches x 2 head-groups (6 heads each).  Each core runs a
flash-attention-style Bass/Tile kernel over its (batch, head-group) shard:

  qT/kT: [384, S] feature-major projections from xT (q pre-scaled by hd^-0.5)
  v:     [S, 6*65] natural layout, a ones column appended per head
  scoresT[k, q] = k . q (contract hd=64, head pairs row-packed on the PE array)
  mask folded in as -1e5*maskT via identity-matmul on PE / in-place DVE add
  pT = exp(scoresT) on the scalar engine (psum -> sbuf)
  avT[65, 512] accumulated over key chunks; row 64 = softmax denominator (ones col)
  attn_outT = avT[0:64] * recip(denominator)  (partition-broadcast on gpsimd)
  y_partial = attn_outT.T @ w_projT slice  (row-parallel output projection)

Host-side: transposes/slices the weights per core, converts the mask to
pre-scaled bf16, sums the two per-batch partials, and adds b_proj.
"""

from contextlib import ExitStack

import numpy as np

import concourse.tile as tile
from concourse import bacc, mybir
from concourse.bass_utils import run_bass_kernel_spmd

F32 = mybir.dt.float32
F32R = mybir.dt.float32r
BF16 = mybir.dt.bfloat16

MASK_NEG = -100000.0
B, S, C, H = 4, 2048, 768, 12
HD = 64
H_PER_CORE = 6
D_CORE = H_PER_CORE * HD  # 384
QBLK = 512
GRP = 2
N_CORES = 8
MASK_DVE_FRAC = 1.0


def _r(ap):
    return ap.bitcast(F32R)


def _build_kernel(mask_dve_frac=MASK_DVE_FRAC):
    nc = bacc.Bacc(
        trn_type="TRN2", target_bir_lowering=False, debug=False, num_devices=N_CORES
    )
    KC = S // 128
    QB = S // QBLK
    NB = S // QBLK
    ST = S // 128
    groups = []
    for half in range(KC // 8):
        base = half * 8
        for g0, gs in ((0, 2), (2, 2), (4, 2), (6, 2)):
            groups.append((base + g0, gs))

    xT = nc.dram_tensor("xT", [C, S], F32R, kind="ExternalInput").ap()
    wq = nc.dram_tensor("wq", [C, D_CORE], F32R, kind="ExternalInput").ap()
    wk = nc.dram_tensor("wk", [C, D_CORE], F32R, kind="ExternalInput").ap()
    wv = nc.dram_tensor("wv", [C, D_CORE], F32R, kind="ExternalInput").ap()
    wproj = nc.dram_tensor("wproj", [D_CORE, C], F32R, kind="ExternalInput").ap()
    vones = nc.dram_tensor("vones", [128, S // 128 * H_PER_CORE], F32R, kind="ExternalInput").ap()
    maskT = nc.dram_tensor("maskT", [S, S], BF16, kind="ExternalInput").ap()
    ident = nc.dram_tensor("ident", [128, 128], BF16, kind="ExternalInput").ap()
    y = nc.dram_tensor("y", [S, C], F32, kind="ExternalOutput").ap()

    with tile.TileContext(nc) as tc, ExitStack() as ctx:
        consts = ctx.enter_context(tc.tile_pool(name="consts", bufs=1))
        qkv_pool = ctx.enter_context(tc.tile_pool(name="qkv", bufs=1))

        ident_sb = consts.tile([128, 128], BF16)
        nc.sync.dma_start(ident_sb[:], ident[:])
        wproj_sb = consts.tile([128, 3, C], F32R)
        nc.sync.dma_start(wproj_sb[:], wproj.rearrange("(t p) o -> p t o", p=128))

        qT_sb = qkv_pool.tile([128, 3, S], F32R)
        kT_sb = qkv_pool.tile([128, 3, S], F32R)
        vaug_sb = qkv_pool.tile([128, ST, H_PER_CORE * (HD + 1)], F32R)
        attn_sb = qkv_pool.tile([128, 3, S], F32R)

        # ones columns (softmax denominator) come from DRAM
        vaug_ones = vaug_sb.rearrange("p st (h u) -> p st h u", u=HD + 1)[:, :, :, HD]
        nc.sync.dma_start(
            vaug_ones, vones.rearrange("p (st h) -> p st h", h=H_PER_CORE)
        )

        # ---------------- phase 1: qkv projections ----------------
        with ExitStack() as p1:
            wpool = p1.enter_context(tc.tile_pool(name="w1", bufs=1))
            xpool = p1.enter_context(tc.tile_pool(name="x1", bufs=1))
            ps1 = p1.enter_context(tc.tile_pool(name="ps1", bufs=3, space="PSUM"))
            psv1 = p1.enter_context(tc.tile_pool(name="psv1", bufs=2, space="PSUM"))

            wq_sb = wpool.tile([128, 6, D_CORE], F32R)
            wk_sb = wpool.tile([128, 6, D_CORE], F32R)
            wv_sb = wpool.tile([128, 6, D_CORE], F32R)
            for w_ap, w_sb in ((wq, wq_sb), (wk, wk_sb), (wv, wv_sb)):
                nc.sync.dma_start(
                    w_sb[:], w_ap.rearrange("(t p) d -> p t d", p=128)
                )
            xT_sb = xpool.tile([128, 6, S], F32R)
            nc.sync.dma_start(xT_sb[:], xT.rearrange("(t p) s -> p t s", p=128))

            cp_i = 0
            for w_sb, dst in ((wq_sb, qT_sb), (wk_sb, kT_sb)):
                for m in range(3):
                    for nb in range(NB):
                        ps = ps1.tile([128, QBLK], F32, tag="psqk", name="psqk")
                        for k in range(6):
                            nc.tensor.matmul(
                                ps[:],
                                w_sb[:, k, m * 128 : (m + 1) * 128],
                                xT_sb[:, k, nb * QBLK : (nb + 1) * QBLK],
                                start=(k == 0),
                                stop=(k == 5),
                            )
                        dst_ap = dst[:, m, nb * QBLK : (nb + 1) * QBLK]
                        if cp_i % 2 == 0:
                            nc.vector.tensor_copy(dst_ap, ps[:])
                        else:
                            nc.scalar.copy(dst_ap, ps[:])
                        cp_i += 1

            for st in range(ST):
                psv = psv1.tile([128, D_CORE], F32, tag="psv", name="psv")
                for k in range(6):
                    nc.tensor.matmul(
                        psv[:],
                        xT_sb[:, k, st * 128 : (st + 1) * 128],
                        wv_sb[:, k, :],
                        start=(k == 0),
                        stop=(k == 5),
                    )
                for h in range(H_PER_CORE):
                    dst = vaug_sb[:, st, h * (HD + 1) : h * (HD + 1) + HD]
                    src = psv[:, h * HD : (h + 1) * HD]
                    if h % 2 == 0:
                        nc.vector.tensor_copy(dst, src)
                    else:
                        nc.scalar.copy(dst, src)

        # ---------------- phase 2: attention ----------------
        with ExitStack() as p2:
            mpool = p2.enter_context(tc.tile_pool(name="mask", bufs=4))
            ppool = p2.enter_context(tc.tile_pool(name="pT", bufs=5))
            dpool = p2.enter_context(tc.tile_pool(name="div", bufs=2))
            bpool = p2.enter_context(tc.tile_pool(name="bcast", bufs=2))
            apool = p2.enter_context(tc.tile_pool(name="avsb", bufs=1))
            ps_s = p2.enter_context(tc.tile_pool(name="ps_s", bufs=3, space="PSUM"))
            ps_av = p2.enter_context(tc.tile_pool(name="ps_av", bufs=2, space="PSUM"))

            mask_idx = 0
            maskT_r = maskT.rearrange("(kc p) q -> p kc q", p=128)
            mask_cache = {}

            def load_mask(qb_i):
                halves = []
                for half in range(KC // 8):
                    mh = mpool.tile([128, 8, QBLK], BF16, tag="mask", name="mask_h")
                    nc.sync.dma_start(
                        mh[:],
                        maskT_r[
                            :,
                            half * 8 : (half + 1) * 8,
                            qb_i * QBLK : (qb_i + 1) * QBLK,
                        ],
                    )
                    halves.append(mh)
                return halves

            mask_cache[0] = load_mask(0)
            for qb in range(QB):
                if qb + 1 < QB:
                    mask_cache[qb + 1] = load_mask(qb + 1)
                mask_halves = mask_cache.pop(qb)
                dstack = dpool.tile([H_PER_CORE, QBLK], F32, tag="dstack", name="dstack")
                recip = dpool.tile([H_PER_CORE, QBLK], F32, tag="recip", name="recip")
                av_all = apool.tile([HD + 1, H_PER_CORE, QBLK], F32, tag="av_all", name="av_all")
                av_keep = []

                for hp in range(3):
                    hA, hB = 2 * hp, 2 * hp + 1
                    av = [
                        ps_av.tile([HD + 1, QBLK], F32, tag="av", name=f"av{hp}a"),
                        ps_av.tile([HD + 1, QBLK], F32, tag="av", name=f"av{hp}b"),
                    ]
                    for (g0, gs) in groups:
                        mh = mask_halves[g0 // 8]
                        moff = g0 % 8
                        sc = [
                            ps_s.tile([128, GRP, QBLK], F32, tag="sc", name="scA"),
                            ps_s.tile([128, GRP, QBLK], F32, tag="sc", name="scB"),
                        ]
                        dve_heads = []
                        for i, h in ((0, hA), (1, hB)):
                            on_dve = (mask_idx % 10) < int(round(mask_dve_frac * 10))
                            mask_idx += 1
                            if on_dve:
                                dve_heads.append(i)
                            else:
                                for c in range(gs):
                                    nc.tensor.matmul(
                                        sc[i][:, c, :],
                                        ident_sb[:],
                                        mh[:, moff + c, :],
                                        start=True,
                                        stop=False,
                                    )
                            row0 = (h % 2) * HD
                            for c in range(gs):
                                kc = g0 + c
                                nc.tensor.matmul(
                                    sc[i][:, c, :],
                                    kT_sb[
                                        row0 : row0 + HD,
                                        h // 2,
                                        kc * 128 : (kc + 1) * 128,
                                    ],
                                    qT_sb[
                                        row0 : row0 + HD,
                                        h // 2,
                                        qb * QBLK : (qb + 1) * QBLK,
                                    ],
                                    start=on_dve,
                                    stop=True,
                                    tile_position=(row0, 0),
                                )
                        for i in dve_heads:
                            nc.vector.tensor_add(
                                sc[i][:, :gs, :],
                                sc[i][:, :gs, :],
                                mh[:, moff : moff + gs, :],
                            )

                        for i, h in ((0, hA), (1, hB)):
                            pT = ppool.tile([128, GRP, QBLK], F32R, tag="pT", name="pT")
                            nc.scalar.activation(
                                pT[:, :gs, :],
                                sc[i][:, :gs, :],
                                mybir.ActivationFunctionType.Exp,
                            )
                            for c in range(gs):
                                kc = g0 + c
                                nc.tensor.matmul(
                                    av[i][:],
                                    vaug_sb[:, kc, h * (HD + 1) : (h + 1) * (HD + 1)],
                                    _r(pT[:, c, :]),
                                    start=(kc == 0),
                                    stop=(kc == KC - 1),
                                )

                    for i, h in ((0, hA), (1, hB)):
                        nc.vector.tensor_copy(av_all[:, h, :], av[i][:])
                        av_keep.append(h)

                # gather all 6 denominator rows in one partition-shift DMA
                nc.gpsimd.dma_start(dstack[:], av_all[HD : HD + 1, :, :])
                nc.vector.reciprocal_approx_fast(recip[:], dstack[:])
                r6 = bpool.tile([1, H_PER_CORE, QBLK], F32, tag="r6", name="r6", bufs=1)
                nc.gpsimd.dma_start(r6[:], recip[:])
                tmp_all = bpool.tile(
                    [HD, 3, QBLK], F32R, tag="tmp_all", name="tmp_all", bufs=1
                )
                for h in av_keep:
                    bc = bpool.tile([HD, QBLK], F32, tag="bc", name="bc")
                    nc.gpsimd.partition_broadcast(bc[:], r6[:, h, :])
                    if h % 2 == 0:
                        dst = attn_sb[:HD, h // 2, qb * QBLK : (qb + 1) * QBLK]
                        nc.vector.tensor_mul(dst, av_all[:HD, h, :], bc[:])
                    else:
                        nc.vector.tensor_mul(
                            tmp_all[:, h // 2, :], av_all[:HD, h, :], bc[:]
                        )
                nc.gpsimd.dma_start(
                    attn_sb[HD:128, :, qb * QBLK : (qb + 1) * QBLK], tmp_all[:]
                )

        # ---------------- phase 3: output projection ----------------
        with ExitStack() as p3:
            ypool = p3.enter_context(tc.tile_pool(name="y", bufs=3))
            ps_y = p3.enter_context(tc.tile_pool(name="ps_y", bufs=4, space="PSUM"))
            y_r = y.rearrange("(st p) o -> st p o", p=128)
            for st in range(ST):
                y_sb = ypool.tile([128, C], F32, tag="ysb", name="y_sb")
                for nb2 in range(2):
                    ps = ps_y.tile([128, 384], F32, tag="psy", name="psy")
                    for k3 in range(3):
                        nc.tensor.matmul(
                            ps[:],
                            attn_sb[:, k3, st * 128 : (st + 1) * 128],
                            wproj_sb[:, k3, nb2 * 384 : (nb2 + 1) * 384],
                            start=(k3 == 0),
                            stop=(k3 == 2),
                        )
                    if nb2 == 0:
                        nc.vector.tensor_copy(y_sb[:, :384], ps[:])
                    else:
                        nc.scalar.copy(y_sb[:, 384:], ps[:])
                nc.sync.dma_start(y_r[st], y_sb[:])

    nc.compile()
    return nc


def _prep_core_inputs(x, mask, w_qkv, w_proj, core):
    import ml_dtypes

    b, g = core // 2, core % 2
    scale = HD ** -0.5
    s0, s1 = 384 * g, 384 * (g + 1)
    return {
        "xT": np.ascontiguousarray(x[b].T),
        "wq": np.ascontiguousarray((w_qkv[s0:s1, :] * scale).T),
        "wk": np.ascontiguousarray(w_qkv[C + s0 : C + s1, :].T),
        "wv": np.ascontiguousarray(w_qkv[2 * C + s0 : 2 * C + s1, :].T),
        "wproj": np.ascontiguousarray(w_proj[:, s0:s1].T),
        "maskT": np.array([0.0, MASK_NEG], dtype=ml_dtypes.bfloat16)[mask[b].T],
        "ident": np.eye(128, dtype=ml_dtypes.bfloat16),
        "vones": np.ones((128, S // 128 * H_PER_CORE), dtype=np.float32),
    }


_NC_CACHE = {}


def get_nc():
    if "nc" not in _NC_CACHE:
        _NC_CACHE["nc"] = _build_kernel()
    return _NC_CACHE["nc"]


def _build_runner(nc):
    """Reusable jitted shard_map callable over the 8 cores (mirrors
    bass2jax.run_bass_via_pjrt but cacheable across calls)."""
    import jax
    from jax.experimental.shard_map import shard_map
    from jax.sharding import Mesh, PartitionSpec

    from concourse.bass2jax import (
        _bass_exec_p,
        install_neuronx_cc_hook,
        partition_id_tensor,
    )

    install_neuronx_cc_hook()
    partition_name = nc.partition_id_tensor.name if nc.partition_id_tensor else None
    in_names, out_names, out_avals, zero_outs = [], [], [], []
    for alloc in nc.m.functions[0].allocations:
        if not isinstance(alloc, mybir.MemoryLocationSet):
            continue
        name = alloc.memorylocations[0].name
        if alloc.kind == "ExternalInput":
            if name != partition_name:
                in_names.append(name)
        elif alloc.kind == "ExternalOutput":
            out_names.append(name)
            shape = tuple(alloc.tensor_shape)
            dtype = mybir.dt.np(alloc.dtype)
            out_avals.append(jax.core.ShapedArray(shape, dtype))
            zero_outs.append(np.zeros(shape, dtype))
    n_params = len(in_names)
    all_in_names = list(in_names) + list(out_names)
    if partition_name is not None:
        all_in_names.append(partition_name)

    def _body(*args):
        operands = list(args)
        if partition_name is not None:
            operands.append(partition_id_tensor())
        outs = _bass_exec_p.bind(
            *operands,
            out_avals=tuple(out_avals),
            in_names=tuple(all_in_names),
            out_names=tuple(out_names),
            lowering_input_output_aliases=(),
            sim_require_finite=True,
            sim_require_nnan=True,
            nc=nc,
        )
        return tuple(outs)

    n_cores = nc.num_devices
    devices = jax.devices()[:n_cores]
    mesh = Mesh(np.asarray(devices), ("core",))
    in_specs = (PartitionSpec("core"),) * (n_params + len(out_names))
    out_specs = (PartitionSpec("core"),) * len(out_names)
    fn = jax.jit(
        shard_map(
            _body, mesh=mesh, in_specs=in_specs, out_specs=out_specs, check_rep=False
        ),
        keep_unused=True,
    )
    return fn, in_names, out_names, zero_outs


_RUNNER_CACHE = {}


def get_runner(nc, in_maps):
    """Return (fn, dev_args) for repeated dispatch of `nc` with `in_maps`."""
    import jax
    from jax.sharding import Mesh, NamedSharding, PartitionSpec

    key = id(nc)
    if key not in _RUNNER_CACHE:
        _RUNNER_CACHE[key] = _build_runner(nc)
    fn, in_names, out_names, zero_outs = _RUNNER_CACHE[key]
    n_cores = nc.num_devices
    mesh = Mesh(np.asarray(jax.devices()[:n_cores]), ("core",))
    shard = NamedSharding(mesh, PartitionSpec("core"))
    concat_in = [
        np.concatenate([np.asarray(in_maps[c][n]) for c in range(n_cores)], axis=0)
        for n in in_names
    ]
    dev_in = [jax.device_put(a, shard) for a in concat_in]
    zkey = ("zeros", key)
    if zkey not in _RUNNER_CACHE:
        concat_zeros = [
            np.zeros((n_cores * z.shape[0], *z.shape[1:]), z.dtype) for z in zero_outs
        ]
        _RUNNER_CACHE[zkey] = [jax.device_put(a, shard) for a in concat_zeros]
    return fn, dev_in + _RUNNER_CACHE[zkey]


def run_cached(nc, in_maps):
    """Execute via the cached runner; returns per-core result dicts."""
    fn, dev_args = get_runner(nc, in_maps)
    out_arrs = fn(*dev_args)
    _, _, out_names, zero_outs = _RUNNER_CACHE[id(nc)]
    n_cores = nc.num_devices
    fetched = [
        np.asarray(a).reshape(n_cores, *zero_outs[i].shape)
        for i, a in enumerate(out_arrs)
    ]
    return [
        {name: fetched[i][c] for i, name in enumerate(out_names)}
        for c in range(n_cores)
    ]


def make_in_maps(x, mask, w_qkv, w_proj):
    return [_prep_core_inputs(x, mask, w_qkv, w_proj, c) for c in range(N_CORES)]


def combine(results, b_proj):
    outs = []
    for b in range(B):
        outs.append(results[2 * b]["y"] + results[2 * b + 1]["y"] + b_proj[None, :])
    return np.stack(outs).astype(np.float32)


def kernel(x, mask, w_qkv, w_proj, b_proj):
    x = np.asarray(x, dtype=np.float32)
    mask = np.asarray(mask)
    w_qkv = np.asarray(w_qkv, dtype=np.float32)
    w_proj = np.asarray(w_proj, dtype=np.float32)
    b_proj = np.asarray(b_proj, dtype=np.float32)

    nc = get_nc()
    in_maps = make_in_maps(x, mask, w_qkv, w_proj)
    try:
        results = run_cached(nc, in_maps)
    except Exception:
        results = run_bass_kernel_spmd(nc, in_maps, list(range(N_CORES))).results
    return combine(results, b_proj)



# revision 15
# speedup vs baseline: 1.4896x; 1.4896x over previous
"""TRN2 Bass kernel v3: masked MHA block (B=4, S=2048, C=768, H=12).

Sharding: 8 cores = 4 batches x 2 head-groups (6 heads each), collective-free;
host sums the two per-batch row-parallel partials and adds b_proj.

All matmuls bf16 (fp32 PSUM accum) -- fp8 softmax paths lose too much accuracy
because softmax-weight noise does NOT wash out relative to the attention
output.  Keys to speed (all measured on this part):
  - dense back-to-back PE streams run unthrottled at 2.4 GHz; sparse streams
    drop to the HAM cold state (half rate), so phase 2 is structured so the
    PE always has runnable matmuls (3 score-psum bufs, 2 av bufs, head-pair
    interleaving).
  - scores: two heads quadrant-packed via tile_position (109 ns/MM measured).
  - AV: 64-key subchunks packed the same way, ones column rides as softmax
    denominator row 64.
  - mask: additive fp8-DR identity matmul into the scores psum for MASK_PE_QBS
    q-blocks; multiplicative bf16 keep-mask on DVE (2x mode) for the rest.
  - exp: ACT activation(Exp)->bf16 for most (h, qb) units; DVE schraudolph
    tensor_scalar -> int16 bitcast bf16 for DVE_UNITS to balance engines.
"""

import math
from contextlib import ExitStack

import numpy as np

import concourse.tile as tile
from concourse import bacc, mybir
from concourse.bass_utils import run_bass_kernel_spmd

F32 = mybir.dt.float32
BF16 = mybir.dt.bfloat16
FP8E4 = mybir.dt.float8e4
FP8E5 = mybir.dt.float8e5
I16 = mybir.dt.int16
DR = mybir.MatmulPerfMode.DoubleRow

MASK_NEG = -57344.0
B, S, C, H = 4, 2048, 768, 12
HD = 64
H_PER_CORE = 6
D_CORE = H_PER_CORE * HD  # 384
QBLK = 512
KC = S // 128  # 16
QB = S // QBLK  # 4
ST = S // 128  # 16
N_CORES = 8
VSTRIDE = 80  # per-head vaug stride (65 used)

EXP_BIAS = -3.5  # only the schraudolph path bakes this in; cancels in softmax
A16 = 128.0 / math.log(2.0)
B16 = 16256.0 + A16 * EXP_BIAS - 8.1

import os

# (head, qb) units where exp runs on DVE (schraudolph) instead of ACT
if os.environ.get("V3_ACT_ONLY"):
    DVE_UNITS = set()
else:
    DVE_UNITS = {(h, qb) for h in (4, 5) for qb in range(QB)}
# q-blocks whose mask is folded on the PE (fp8-DR identity); others use a
# multiplicative bf16 keep-mask on DVE after exp
if os.environ.get("V3_MASK_ALL_PE"):
    MASK_PE_QBS = set(range(QB))
else:
    MASK_PE_QBS = {0}


def _build_kernel():
    nc = bacc.Bacc(
        trn_type="TRN2", target_bir_lowering=False, debug=False, num_devices=N_CORES
    )

    xb_d = nc.dram_tensor("xb", [128, 6, S], BF16, kind="ExternalInput").ap()
    wq_d = nc.dram_tensor("wqb", [128, 6, D_CORE], BF16, kind="ExternalInput").ap()
    wk_d = nc.dram_tensor("wkb", [128, 6, D_CORE], BF16, kind="ExternalInput").ap()
    wv_d = nc.dram_tensor("wvb", [128, 6, D_CORE], BF16, kind="ExternalInput").ap()
    maske_d = nc.dram_tensor(
        "maske5", [128, KC, 2, S], FP8E5, kind="ExternalInput"
    ).ap()
    keep_d = nc.dram_tensor("keepb", [128, KC, S], BF16, kind="ExternalInput").ap()
    id8_d = nc.dram_tensor("ident8", [128, 2, 128], FP8E4, kind="ExternalInput").ap()
    wproj_d = nc.dram_tensor("wproj", [128, 3, C], BF16, kind="ExternalInput").ap()
    y_d = nc.dram_tensor("y", [S, C], F32, kind="ExternalOutput").ap()

    with tile.TileContext(nc) as tc, ExitStack() as ctx:
        consts = ctx.enter_context(tc.tile_pool(name="consts", bufs=1))
        main = ctx.enter_context(tc.tile_pool(name="main", bufs=1))

        id8_sb = consts.tile([128, 2, 128], FP8E4, tag="id8", name="id8")
        nc.sync.dma_start(id8_sb[:], id8_d[:])
        wproj_sb = consts.tile([128, 3, C], BF16, tag="wproj", name="wproj")
        nc.sync.dma_start(wproj_sb[:], wproj_d[:])

        qT_sb = main.tile([128, 3, S], BF16, tag="qT", name="qT")
        kT_sb = main.tile([128, 3, S], BF16, tag="kT", name="kT")
        vaug = main.tile([128, KC, H_PER_CORE * VSTRIDE], BF16, tag="vaug", name="vaug")
        attn_sb = main.tile([128, 3, S], BF16, tag="attn", name="attn")

        vaug_h = vaug.rearrange("p kc (h u) -> p kc h u", u=VSTRIDE)
        nc.gpsimd.memset(vaug_h[:, :, :, HD : HD + 1], 1.0)
        if os.environ.get("V3_NO_AV") or os.environ.get("V3_NO_NORM"):
            nc.gpsimd.memset(attn_sb[:], 0.0)

        # ---------------- phase 1: QKV projections (bf16) ----------
        with ExitStack() as p1:
            xpool = p1.enter_context(tc.tile_pool(name="x1", bufs=1))
            wpool = p1.enter_context(tc.tile_pool(name="w1", bufs=1))
            ps1 = p1.enter_context(tc.tile_pool(name="ps1", bufs=3, space="PSUM"))
            psv1 = p1.enter_context(tc.tile_pool(name="psv1", bufs=2, space="PSUM"))

            xb_sb = xpool.tile([128, 6, S], BF16, tag="xb", name="xb")
            nc.sync.dma_start(xb_sb[:], xb_d[:])
            w_sbs = []
            for nm, w_ap in (("wq", wq_d), ("wk", wk_d), ("wv", wv_d)):
                w_sb = wpool.tile([128, 6, D_CORE], BF16, tag=nm, name=nm)
                nc.sync.dma_start(w_sb[:], w_ap[:])
                w_sbs.append(w_sb)
            wq_sb, wk_sb, wv_sb = w_sbs

            cp_i = 0
            for w_sb, dst in ((wq_sb, qT_sb), (wk_sb, kT_sb)):
                for m in range(3):
                    for nb in range(QB):
                        ps = ps1.tile([128, QBLK], F32, tag="psqk", name="psqk")
                        for k in range(6):
                            nc.tensor.matmul(
                                ps[:],
                                w_sb[:, k, m * 128 : (m + 1) * 128],
                                xb_sb[:, k, nb * QBLK : (nb + 1) * QBLK],
                                start=(k == 0),
                                stop=(k == 5),
                            )
                        dst_ap = dst[:, m, nb * QBLK : (nb + 1) * QBLK]
                        if cp_i % 2 == 0:
                            nc.vector.tensor_copy(dst_ap, ps[:])
                        else:
                            nc.scalar.copy(dst_ap, ps[:])
                        cp_i += 1

            for st in range(ST):
                psv = psv1.tile([128, D_CORE], F32, tag="psv", name="psv")
                for k in range(6):
                    nc.tensor.matmul(
                        psv[:],
                        xb_sb[:, k, st * 128 : (st + 1) * 128],
                        wv_sb[:, k, :],
                        start=(k == 0),
                        stop=(k == 5),
                    )
                dst = vaug_h[:, st, :, 0:HD]
                src = psv.rearrange("p (h d) -> p h d", d=HD)
                if st % 2 == 0:
                    nc.vector.tensor_copy(dst, src)
                else:
                    nc.scalar.copy(dst, src)

        # ---------------- phase 2: attention ----------------
        with ExitStack() as p2:
            mpool = p2.enter_context(tc.tile_pool(name="mask", bufs=2))
            ppool = p2.enter_context(tc.tile_pool(name="pT", bufs=3))
            dpool = p2.enter_context(tc.tile_pool(name="div", bufs=2))
            bpool = p2.enter_context(tc.tile_pool(name="bcast", bufs=2))
            ps_s = p2.enter_context(tc.tile_pool(name="ps_s", bufs=3, space="PSUM"))
            ps_av = p2.enter_context(tc.tile_pool(name="ps_av", bufs=2, space="PSUM"))

            mask_cache = {}

            def load_mask(qb_i):
                if qb_i in MASK_PE_QBS:
                    mh = mpool.tile([128, KC, 2, QBLK], FP8E5, tag="mask", name="maske")
                    nc.sync.dma_start(
                        mh[:], maske_d[:, :, :, qb_i * QBLK : (qb_i + 1) * QBLK]
                    )
                else:
                    mh = mpool.tile([128, KC, QBLK], BF16, tag="mask", name="maskk")
                    nc.sync.dma_start(
                        mh[:], keep_d[:, :, qb_i * QBLK : (qb_i + 1) * QBLK]
                    )
                return mh

            mask_cache[0] = load_mask(0)
            for qb in range(QB):
                if qb + 1 < QB:
                    mask_cache[qb + 1] = load_mask(qb + 1)
                mask_sb = mask_cache.pop(qb)
                mask_on_pe = qb in MASK_PE_QBS
                tmp_all = bpool.tile(
                    [HD, 3, QBLK], F32, tag="tmp_all", name="tmp_all", bufs=1
                )

                for hp in range(3):
                    hA, hB = 2 * hp, 2 * hp + 1
                    pTs = {}
                    for h in (hA, hB):
                        pTs[h] = ppool.tile([128, KC, QBLK], BF16, tag="pT", name="pT")
                    # scores + mask + exp, kc-pair at a time, heads interleaved
                    for kcp in range(KC // 2):
                        scs = {}
                        for h in (hA, hB):
                            scs[h] = ps_s.tile(
                                [128, 2, QBLK], F32, tag="sc", name="sc"
                            )
                        for c in range(2):
                            kc = 2 * kcp + c
                            for h in (hA, hB):
                                row0 = (h % 2) * HD
                                nc.tensor.matmul(
                                    scs[h][:, c, :],
                                    kT_sb[
                                        row0 : row0 + HD, hp, kc * 128 : (kc + 1) * 128
                                    ],
                                    qT_sb[
                                        row0 : row0 + HD,
                                        hp,
                                        qb * QBLK : (qb + 1) * QBLK,
                                    ],
                                    start=True,
                                    stop=not mask_on_pe,
                                    tile_position=(row0, 0),
                                )
                            if mask_on_pe:
                                for h in (hA, hB):
                                    nc.tensor.matmul(
                                        scs[h][:, c, :],
                                        id8_sb[:],
                                        mask_sb[:, kc, :, :],
                                        start=False,
                                        stop=True,
                                        perf_mode=DR,
                                    )
                        for h in (hA, hB):
                            pslab = pTs[h][:, 2 * kcp : 2 * kcp + 2, :]
                            if (h, qb) in DVE_UNITS:
                                nc.vector.tensor_scalar(
                                    pslab.bitcast(I16),
                                    scs[h][:],
                                    A16,
                                    B16,
                                    mybir.AluOpType.mult,
                                    mybir.AluOpType.add,
                                )
                            else:
                                nc.scalar.activation(
                                    pslab,
                                    scs[h][:],
                                    mybir.ActivationFunctionType.Exp,
                                )
                            if not mask_on_pe:
                                nc.vector.tensor_mul(
                                    pslab,
                                    pslab,
                                    mask_sb[:, 2 * kcp : 2 * kcp + 2, :],
                                )
                    if os.environ.get("V3_NO_AV"):
                        continue
                    # AV, 64-key subchunks quadrant-packed across the pair
                    avs = {
                        hA: ps_av.tile([HD + 1, QBLK], F32, tag="av", name="avA"),
                        hB: ps_av.tile([HD + 1, QBLK], F32, tag="av", name="avB"),
                    }
                    if not os.environ.get("V3_AV_PACKED"):
                        for h in (hA, hB):
                            for kc in range(KC):
                                nc.tensor.matmul(
                                    avs[h][:],
                                    vaug_h[:, kc, h, 0 : HD + 1],
                                    pTs[h][:, kc, :],
                                    start=(kc == 0),
                                    stop=(kc == KC - 1),
                                )
                    else:
                        n_mm = 2 * KC
                        for kc in range(KC):
                            for sub in range(2):
                                r0 = sub * HD
                                i_mm = 2 * kc + sub
                                for h, rr in ((hA, r0), (hB, HD - r0)):
                                    nc.tensor.matmul(
                                        avs[h][:],
                                        vaug_h[
                                            rr : rr + HD, kc, h, 0 : HD + 1
                                        ],
                                        pTs[h][rr : rr + HD, kc, :],
                                        start=(i_mm == 0),
                                        stop=(i_mm == n_mm - 1),
                                        tile_position=(rr, 0),
                                    )
                    if os.environ.get("V3_NO_NORM"):
                        continue
                    # normalize: denominator rows -> partition 0 via DMA,
                    # reciprocal, partition-broadcast, scale
                    den = dpool.tile([65, 2, QBLK], F32, tag="den", name="den")
                    for i, h in ((0, hA), (1, hB)):
                        nc.scalar.copy(den[64:65, i, :], avs[h][HD : HD + 1, :])
                    r2 = dpool.tile([1, 2, QBLK], F32, tag="r2", name="r2")
                    nc.gpsimd.dma_start(r2[:], den[64:65, :, :])
                    rec2 = dpool.tile([1, 2, QBLK], F32, tag="rec2", name="rec2")
                    nc.vector.reciprocal_approx_fast(rec2[:], r2[:])
                    for i, h in ((0, hA), (1, hB)):
                        bc = bpool.tile([HD, QBLK], F32, tag="bc", name="bc")
                        nc.gpsimd.partition_broadcast(bc[:], rec2[:, i, :])
                        if h % 2 == 0:
                            dst = attn_sb[:HD, hp, qb * QBLK : (qb + 1) * QBLK]
                            nc.vector.tensor_mul(dst, avs[h][:HD, :], bc[:])
                        else:
                            nc.vector.tensor_mul(
                                tmp_all[:, hp, :], avs[h][:HD, :], bc[:]
                            )
                if not (os.environ.get("V3_NO_AV") or os.environ.get("V3_NO_NORM")):
                    nc.gpsimd.dma_start(
                        attn_sb[HD:128, :, qb * QBLK : (qb + 1) * QBLK], tmp_all[:]
                    )

        # ---------------- phase 3: output projection (bf16) ----------------
        with ExitStack() as p3:
            ypool = p3.enter_context(tc.tile_pool(name="y", bufs=3))
            ps_y = p3.enter_context(tc.tile_pool(name="ps_y", bufs=4, space="PSUM"))
            y_r = y_d.rearrange("(st p) o -> st p o", p=128)
            for st in range(ST):
                y_sb = ypool.tile([128, C], F32, tag="ysb", name="y_sb")
                for nb2 in range(2):
                    ps = ps_y.tile([128, 384], F32, tag="psy", name="psy")
                    for k3 in range(3):
                        nc.tensor.matmul(
                            ps[:],
                            attn_sb[:, k3, st * 128 : (st + 1) * 128],
                            wproj_sb[:, k3, nb2 * 384 : (nb2 + 1) * 384],
                            start=(k3 == 0),
                            stop=(k3 == 2),
                        )
                    if nb2 == 0:
                        nc.vector.tensor_copy(y_sb[:, :384], ps[:])
                    else:
                        nc.scalar.copy(y_sb[:, 384:], ps[:])
                nc.sync.dma_start(y_r[st], y_sb[:])

    nc.compile()
    return nc


def _prep_core_inputs(x, mask, w_qkv, w_proj, core):
    import ml_dtypes

    f8 = ml_dtypes.float8_e4m3
    f8e5 = ml_dtypes.float8_e5m2
    bf16 = ml_dtypes.bfloat16

    b, g = core // 2, core % 2
    s0, s1 = D_CORE * g, D_CORE * (g + 1)

    def cmajor(a):  # [C, n] -> [128, 6, n]
        return np.ascontiguousarray(a.reshape(6, 128, a.shape[1]).transpose(1, 0, 2))

    xT = np.ascontiguousarray(x[b].T)  # [C, S]
    wq = w_qkv[s0:s1, :].T * (HD ** -0.5)
    wk = w_qkv[C + s0 : C + s1, :].T
    wv = w_qkv[2 * C + s0 : 2 * C + s1, :].T

    maskT = mask[b].T  # [S(key), S(q)]
    m2 = np.zeros((128, KC, 2, S), dtype=f8e5)
    m2[:, :, 0, :] = (
        np.array([0.0, MASK_NEG], dtype=np.float32)[maskT]
        .reshape(KC, 128, S)
        .transpose(1, 0, 2)
        .astype(f8e5)
    )
    keep = (
        np.array([1.0, 0.0], dtype=np.float32)[maskT]
        .reshape(KC, 128, S)
        .transpose(1, 0, 2)
        .astype(bf16)
    )

    id8 = np.zeros((128, 2, 128), dtype=f8)
    id8[:, 0, :] = np.eye(128, dtype=np.float32).astype(f8)

    wproj = np.ascontiguousarray(w_proj[:, s0:s1].T)  # [384, C]
    wproj2 = wproj.reshape(3, 128, C).transpose(1, 0, 2)

    return {
        "xb": cmajor(xT).astype(bf16),
        "wqb": cmajor(np.ascontiguousarray(wq)).astype(bf16),
        "wkb": cmajor(np.ascontiguousarray(wk)).astype(bf16),
        "wvb": cmajor(np.ascontiguousarray(wv)).astype(bf16),
        "maske5": m2,
        "keepb": np.ascontiguousarray(keep),
        "ident8": id8,
        "wproj": np.ascontiguousarray(wproj2).astype(bf16),
    }


_NC_CACHE = {}


def get_nc():
    if "nc" not in _NC_CACHE:
        _NC_CACHE["nc"] = _build_kernel()
    return _NC_CACHE["nc"]


def _build_runner(nc):
    """Reusable jitted shard_map callable over the 8 cores."""
    import jax
    from jax.experimental.shard_map import shard_map
    from jax.sharding import Mesh, PartitionSpec

    from concourse.bass2jax import (
        _bass_exec_p,
        install_neuronx_cc_hook,
        partition_id_tensor,
    )

    install_neuronx_cc_hook()
    partition_name = nc.partition_id_tensor.name if nc.partition_id_tensor else None
    in_names, out_names, out_avals, zero_outs = [], [], [], []
    for alloc in nc.m.functions[0].allocations:
        if not isinstance(alloc, mybir.MemoryLocationSet):
            continue
        name = alloc.memorylocations[0].name
        if alloc.kind == "ExternalInput":
            if name != partition_name:
                in_names.append(name)
        elif alloc.kind == "ExternalOutput":
            out_names.append(name)
            shape = tuple(alloc.tensor_shape)
            dtype = mybir.dt.np(alloc.dtype)
            out_avals.append(jax.core.ShapedArray(shape, dtype))
            zero_outs.append(np.zeros(shape, dtype))
    n_params = len(in_names)
    all_in_names = list(in_names) + list(out_names)
    if partition_name is not None:
        all_in_names.append(partition_name)

    def _body(*args):
        operands = list(args)
        if partition_name is not None:
            operands.append(partition_id_tensor())
        outs = _bass_exec_p.bind(
            *operands,
            out_avals=tuple(out_avals),
            in_names=tuple(all_in_names),
            out_names=tuple(out_names),
            lowering_input_output_aliases=(),
            sim_require_finite=True,
            sim_require_nnan=True,
            nc=nc,
        )
        return tuple(outs)

    n_cores = nc.num_devices
    devices = jax.devices()[:n_cores]
    mesh = Mesh(np.asarray(devices), ("core",))
    in_specs = (PartitionSpec("core"),) * (n_params + len(out_names))
    out_specs = (PartitionSpec("core"),) * len(out_names)
    fn = jax.jit(
        shard_map(
            _body, mesh=mesh, in_specs=in_specs, out_specs=out_specs, check_rep=False
        ),
        keep_unused=True,
    )
    return fn, in_names, out_names, zero_outs


_RUNNER_CACHE = {}


def get_runner(nc, in_maps):
    """Return (fn, dev_args) for repeated dispatch of `nc` with `in_maps`."""
    import jax
    from jax.sharding import Mesh, NamedSharding, PartitionSpec

    key = id(nc)
    if key not in _RUNNER_CACHE:
        _RUNNER_CACHE[key] = _build_runner(nc)
    fn, in_names, out_names, zero_outs = _RUNNER_CACHE[key]
    n_cores = nc.num_devices
    mesh = Mesh(np.asarray(jax.devices()[:n_cores]), ("core",))
    shard = NamedSharding(mesh, PartitionSpec("core"))
    concat_in = [
        np.concatenate([np.asarray(in_maps[c][n]) for c in range(n_cores)], axis=0)
        for n in in_names
    ]
    dev_in = [jax.device_put(a, shard) for a in concat_in]
    zkey = ("zeros", key)
    if zkey not in _RUNNER_CACHE:
        concat_zeros = [
            np.zeros((n_cores * z.shape[0], *z.shape[1:]), z.dtype) for z in zero_outs
        ]
        _RUNNER_CACHE[zkey] = [jax.device_put(a, shard) for a in concat_zeros]
    return fn, dev_in + _RUNNER_CACHE[zkey]


def run_cached(nc, in_maps):
    """Execute via the cached runner; returns per-core result dicts."""
    fn, dev_args = get_runner(nc, in_maps)
    out_arrs = fn(*dev_args)
    _, _, out_names, zero_outs = _RUNNER_CACHE[id(nc)]
    n_cores = nc.num_devices
    fetched = [
        np.asarray(a).reshape(n_cores, *zero_outs[i].shape)
        for i, a in enumerate(out_arrs)
    ]
    return [
        {name: fetched[i][c] for i, name in enumerate(out_names)}
        for c in range(n_cores)
    ]


def make_in_maps(x, mask, w_qkv, w_proj):
    return [_prep_core_inputs(x, mask, w_qkv, w_proj, c) for c in range(N_CORES)]


def combine(results, b_proj):
    outs = []
    for b in range(B):
        outs.append(results[2 * b]["y"] + results[2 * b + 1]["y"] + b_proj[None, :])
    return np.stack(outs).astype(np.float32)


def kernel(x, mask, w_qkv, w_proj, b_proj):
    x = np.asarray(x, dtype=np.float32)
    mask = np.asarray(mask)
    w_qkv = np.asarray(w_qkv, dtype=np.float32)
    w_proj = np.asarray(w_proj, dtype=np.float32)
    b_proj = np.asarray(b_proj, dtype=np.float32)

    nc = get_nc()
    in_maps = make_in_maps(x, mask, w_qkv, w_proj)
    try:
        results = run_cached(nc, in_maps)
    except Exception:
        results = run_bass_kernel_spmd(nc, in_maps, list(range(N_CORES))).results
    return combine(results, b_proj)


# revision 17
# speedup vs baseline: 1.6505x; 1.1080x over previous
"""TRN2 Bass kernel v3: masked MHA block (B=4, S=2048, C=768, H=12).

Sharding: 8 cores = 4 batches x 2 head-groups (6 heads each), collective-free;
host sums the two per-batch row-parallel partials and adds b_proj.

All matmuls bf16 (fp32 PSUM accum) -- fp8 softmax paths lose too much accuracy
because softmax-weight noise does NOT wash out relative to the attention
output.  Keys to speed (all measured on this part):
  - dense back-to-back PE streams run unthrottled at 2.4 GHz; sparse streams
    drop to the HAM cold state (half rate), so phase 2 is structured so the
    PE always has runnable matmuls (3 score-psum bufs, 2 av bufs, head-pair
    interleaving).
  - scores: two heads quadrant-packed via tile_position (109 ns/MM measured).
  - AV: 64-key subchunks packed the same way, ones column rides as softmax
    denominator row 64.
  - mask: additive fp8-DR identity matmul into the scores psum for MASK_PE_QBS
    q-blocks; multiplicative bf16 keep-mask on DVE (2x mode) for the rest.
  - exp: ACT activation(Exp)->bf16 for most (h, qb) units; DVE schraudolph
    tensor_scalar -> int16 bitcast bf16 for DVE_UNITS to balance engines.
"""

import math
from contextlib import ExitStack

import numpy as np

import concourse.tile as tile
from concourse import bacc, mybir
from concourse.bass_utils import run_bass_kernel_spmd

F32 = mybir.dt.float32
BF16 = mybir.dt.bfloat16
FP8E4 = mybir.dt.float8e4
FP8E5 = mybir.dt.float8e5
I16 = mybir.dt.int16
DR = mybir.MatmulPerfMode.DoubleRow

MASK_NEG = -57344.0
B, S, C, H = 4, 2048, 768, 12
HD = 64
H_PER_CORE = 6
D_CORE = H_PER_CORE * HD  # 384
QBLK = 512
KC = S // 128  # 16
QB = S // QBLK  # 4
ST = S // 128  # 16
N_CORES = 8
VSTRIDE = 80  # per-head vaug stride (65 used)

EXP_BIAS = -3.5  # only the schraudolph path bakes this in; cancels in softmax
A16 = 128.0 / math.log(2.0)
B16 = 16256.0 + A16 * EXP_BIAS - 8.1

import os

# (head, qb) units where exp runs on DVE (schraudolph) instead of ACT
if os.environ.get("V3_ACT_ONLY"):
    DVE_UNITS = set()
else:
    DVE_UNITS = {(h, qb) for h in (4, 5) for qb in range(QB)} - {
        (4, 0), (5, 0)}
# q-blocks whose mask is folded on the PE (fp8-DR identity); others use a
# multiplicative bf16 keep-mask on DVE after exp
if os.environ.get("V3_MASK_ALL_PE"):
    MASK_PE_QBS = set(range(QB))
elif os.environ.get("V3_MASK_PE_QB0"):
    MASK_PE_QBS = {0}
else:
    MASK_PE_QBS = set()


def _build_kernel():
    nc = bacc.Bacc(
        trn_type="TRN2", target_bir_lowering=False, debug=False, num_devices=N_CORES
    )

    xb_d = nc.dram_tensor("xb", [128, 6, S], BF16, kind="ExternalInput").ap()
    wq_d = nc.dram_tensor("wqb", [128, 6, D_CORE], BF16, kind="ExternalInput").ap()
    wk_d = nc.dram_tensor("wkb", [128, 6, D_CORE], BF16, kind="ExternalInput").ap()
    wv_d = nc.dram_tensor("wvb", [128, 6, D_CORE], BF16, kind="ExternalInput").ap()
    maske_d = nc.dram_tensor(
        "maske5", [128, KC, 2, S], FP8E5, kind="ExternalInput"
    ).ap()
    keep_d = nc.dram_tensor("keepb", [128, KC, S], BF16, kind="ExternalInput").ap()
    id8_d = nc.dram_tensor("ident8", [128, 2, 128], FP8E4, kind="ExternalInput").ap()
    wproj_d = nc.dram_tensor("wproj", [128, 3, C], BF16, kind="ExternalInput").ap()
    y_d = nc.dram_tensor("y", [S, C], F32, kind="ExternalOutput").ap()

    with tile.TileContext(nc) as tc, ExitStack() as ctx:
        consts = ctx.enter_context(tc.tile_pool(name="consts", bufs=1))
        main = ctx.enter_context(tc.tile_pool(name="main", bufs=1))

        id8_sb = consts.tile([128, 2, 128], FP8E4, tag="id8", name="id8")
        nc.sync.dma_start(id8_sb[:], id8_d[:])
        wproj_sb = consts.tile([128, 3, C], BF16, tag="wproj", name="wproj")
        nc.sync.dma_start(wproj_sb[:], wproj_d[:])

        qT_sb = main.tile([128, 3, S], BF16, tag="qT", name="qT")
        kT_sb = main.tile([128, 3, S], BF16, tag="kT", name="kT")
        vaug = main.tile([128, KC, H_PER_CORE * VSTRIDE], BF16, tag="vaug", name="vaug")
        attn_sb = main.tile([128, 3, S], BF16, tag="attn", name="attn")

        vaug_h = vaug.rearrange("p kc (h u) -> p kc h u", u=VSTRIDE)
        nc.gpsimd.memset(vaug_h[:, :, :, HD : HD + 1], 1.0)
        if os.environ.get("V3_NO_AV") or os.environ.get("V3_NO_NORM"):
            nc.gpsimd.memset(attn_sb[:], 0.0)

        # ---------------- phase 1: QKV projections (bf16) ----------
        with ExitStack() as p1:
            xpool = p1.enter_context(tc.tile_pool(name="x1", bufs=1))
            wpool = p1.enter_context(tc.tile_pool(name="w1", bufs=1))
            ps1 = p1.enter_context(tc.tile_pool(name="ps1", bufs=3, space="PSUM"))
            psv1 = p1.enter_context(tc.tile_pool(name="psv1", bufs=2, space="PSUM"))

            xb_sb = xpool.tile([128, 6, S], BF16, tag="xb", name="xb")
            nc.sync.dma_start(xb_sb[:], xb_d[:])
            w_sbs = []
            for nm, w_ap in (("wq", wq_d), ("wk", wk_d), ("wv", wv_d)):
                w_sb = wpool.tile([128, 6, D_CORE], BF16, tag=nm, name=nm)
                nc.sync.dma_start(w_sb[:], w_ap[:])
                w_sbs.append(w_sb)
            wq_sb, wk_sb, wv_sb = w_sbs

            cp_i = 0
            for w_sb, dst in ((wq_sb, qT_sb), (wk_sb, kT_sb)):
                for m in range(3):
                    for nb in range(QB):
                        ps = ps1.tile([128, QBLK], F32, tag="psqk", name="psqk")
                        for k in range(6):
                            nc.tensor.matmul(
                                ps[:],
                                w_sb[:, k, m * 128 : (m + 1) * 128],
                                xb_sb[:, k, nb * QBLK : (nb + 1) * QBLK],
                                start=(k == 0),
                                stop=(k == 5),
                            )
                        dst_ap = dst[:, m, nb * QBLK : (nb + 1) * QBLK]
                        if cp_i % 2 == 0:
                            nc.vector.tensor_copy(dst_ap, ps[:])
                        else:
                            nc.scalar.copy(dst_ap, ps[:])
                        cp_i += 1

            for st in range(ST):
                psv = psv1.tile([128, D_CORE], F32, tag="psv", name="psv")
                for k in range(6):
                    nc.tensor.matmul(
                        psv[:],
                        xb_sb[:, k, st * 128 : (st + 1) * 128],
                        wv_sb[:, k, :],
                        start=(k == 0),
                        stop=(k == 5),
                    )
                dst = vaug_h[:, st, :, 0:HD]
                src = psv.rearrange("p (h d) -> p h d", d=HD)
                if st % 2 == 0:
                    nc.vector.tensor_copy(dst, src)
                else:
                    nc.scalar.copy(dst, src)

        # ---------------- phase 2: attention ----------------
        with ExitStack() as p2:
            mpool = p2.enter_context(tc.tile_pool(name="mask", bufs=2))
            ppool = p2.enter_context(tc.tile_pool(name="pT", bufs=3))
            dpool = p2.enter_context(tc.tile_pool(name="div", bufs=2))
            bpool = p2.enter_context(tc.tile_pool(name="bcast", bufs=2))
            ps_s = p2.enter_context(tc.tile_pool(name="ps_s", bufs=2, space="PSUM"))
            ps_av = p2.enter_context(tc.tile_pool(name="ps_av", bufs=4, space="PSUM"))

            mask_cache = {}

            def load_mask(qb_i):
                if qb_i in MASK_PE_QBS:
                    mh = mpool.tile([128, KC, 2, QBLK], FP8E5, tag="mask", name="maske")
                    nc.sync.dma_start(
                        mh[:], maske_d[:, :, :, qb_i * QBLK : (qb_i + 1) * QBLK]
                    )
                else:
                    mh = mpool.tile([128, KC, QBLK], BF16, tag="mask", name="maskk")
                    nc.sync.dma_start(
                        mh[:], keep_d[:, :, qb_i * QBLK : (qb_i + 1) * QBLK]
                    )
                return mh

            mask_cache[0] = load_mask(0)
            for qb in range(QB):
                if qb + 1 < QB:
                    mask_cache[qb + 1] = load_mask(qb + 1)
                mask_sb = mask_cache.pop(qb)
                mask_on_pe = qb in MASK_PE_QBS
                tmp_all = bpool.tile(
                    [HD, 3, QBLK], F32, tag="tmp_all", name="tmp_all", bufs=1
                )

                for hp in range(3):
                    hA, hB = 2 * hp, 2 * hp + 1
                    pTs = {}
                    for h in (hA, hB):
                        pTs[h] = ppool.tile([128, KC, QBLK], BF16, tag="pT", name="pT")
                    # scores + mask + exp, kc-pair at a time, heads interleaved
                    for kcp in range(KC // 2):
                        scs = {}
                        for h in (hA, hB):
                            scs[h] = ps_s.tile(
                                [128, 2, QBLK], F32, tag="sc", name="sc"
                            )
                        for c in range(2):
                            kc = 2 * kcp + c
                            for h in (hA, hB):
                                row0 = (h % 2) * HD
                                nc.tensor.matmul(
                                    scs[h][:, c, :],
                                    kT_sb[
                                        row0 : row0 + HD, hp, kc * 128 : (kc + 1) * 128
                                    ],
                                    qT_sb[
                                        row0 : row0 + HD,
                                        hp,
                                        qb * QBLK : (qb + 1) * QBLK,
                                    ],
                                    start=True,
                                    stop=not mask_on_pe,
                                    tile_position=(row0, 0),
                                )
                            if mask_on_pe:
                                for h in (hA, hB):
                                    nc.tensor.matmul(
                                        scs[h][:, c, :],
                                        id8_sb[:],
                                        mask_sb[:, kc, :, :],
                                        start=False,
                                        stop=True,
                                        perf_mode=DR,
                                    )
                        for h in (hA, hB):
                            pslab = pTs[h][:, 2 * kcp : 2 * kcp + 2, :]
                            if (h, qb) in DVE_UNITS:
                                nc.vector.tensor_scalar(
                                    pslab.bitcast(I16),
                                    scs[h][:],
                                    A16,
                                    B16,
                                    mybir.AluOpType.mult,
                                    mybir.AluOpType.add,
                                )
                            else:
                                nc.scalar.activation(
                                    pslab,
                                    scs[h][:],
                                    mybir.ActivationFunctionType.Exp,
                                )
                            if not mask_on_pe:
                                nc.vector.tensor_mul(
                                    pslab,
                                    pslab,
                                    mask_sb[:, 2 * kcp : 2 * kcp + 2, :],
                                )
                    if os.environ.get("V3_NO_AV"):
                        continue
                    # AV: 64-key subchunks quadrant-packed across the pair;
                    # each psum tile is written from exactly one quadrant
                    # (low/high halves merged during normalization)
                    if not os.environ.get("V3_AV_PACKED"):
                        avs = {
                            hA: ps_av.tile([HD + 1, QBLK], F32, tag="av", name="avA"),
                            hB: ps_av.tile([HD + 1, QBLK], F32, tag="av", name="avB"),
                        }
                        for h in (hA, hB):
                            for kc in range(KC):
                                nc.tensor.matmul(
                                    avs[h][:],
                                    vaug_h[:, kc, h, 0 : HD + 1],
                                    pTs[h][:, kc, :],
                                    start=(kc == 0),
                                    stop=(kc == KC - 1),
                                )
                        avs2 = None
                    else:
                        avs = {
                            hA: ps_av.tile([HD + 1, QBLK], F32, tag="av", name="avAL"),
                            hB: ps_av.tile([HD + 1, QBLK], F32, tag="av", name="avBL"),
                        }
                        avs2 = {
                            hA: ps_av.tile([HD + 1, QBLK], F32, tag="av", name="avAH"),
                            hB: ps_av.tile([HD + 1, QBLK], F32, tag="av", name="avBH"),
                        }
                        for kc in range(KC):
                            for h, hofs in ((hA, 0), (hB, HD)):
                                rr = hofs  # low half for A first, high for B
                                tgt = avs if rr == 0 else avs2
                                nc.tensor.matmul(
                                    tgt[h][:],
                                    vaug_h[rr : rr + HD, kc, h, 0 : HD + 1],
                                    pTs[h][rr : rr + HD, kc, :],
                                    start=(kc == 0),
                                    stop=(kc == KC - 1),
                                    tile_position=(rr, 0),
                                )
                            for h, hofs in ((hA, HD), (hB, 0)):
                                rr = hofs
                                tgt = avs if rr == 0 else avs2
                                nc.tensor.matmul(
                                    tgt[h][:],
                                    vaug_h[rr : rr + HD, kc, h, 0 : HD + 1],
                                    pTs[h][rr : rr + HD, kc, :],
                                    start=(kc == 0),
                                    stop=(kc == KC - 1),
                                    tile_position=(rr, 0),
                                )
                    if os.environ.get("V3_NO_NORM"):
                        continue
                    # normalize: denominator rows -> partition 0 via DMA,
                    # reciprocal, partition-broadcast, scale
                    den = dpool.tile([65, 2, QBLK], F32, tag="den", name="den")
                    for i, h in ((0, hA), (1, hB)):
                        if avs2 is None:
                            nc.scalar.copy(den[64:65, i, :], avs[h][HD : HD + 1, :])
                        else:
                            nc.vector.tensor_add(
                                den[64:65, i, :],
                                avs[h][HD : HD + 1, :],
                                avs2[h][HD : HD + 1, :],
                            )
                    r2 = dpool.tile([1, 2, QBLK], F32, tag="r2", name="r2")
                    nc.gpsimd.dma_start(r2[:], den[64:65, :, :])
                    rec2 = dpool.tile([1, 2, QBLK], F32, tag="rec2", name="rec2")
                    nc.vector.reciprocal_approx_fast(rec2[:], r2[:])
                    for i, h in ((0, hA), (1, hB)):
                        bc = bpool.tile([HD, QBLK], F32, tag="bc", name="bc")
                        nc.gpsimd.partition_broadcast(bc[:], rec2[:, i, :])
                        if h % 2 == 0:
                            dst = attn_sb[:HD, hp, qb * QBLK : (qb + 1) * QBLK]
                        else:
                            dst = tmp_all[:, hp, :]
                        if avs2 is None:
                            nc.vector.tensor_mul(dst, avs[h][:HD, :], bc[:])
                        else:
                            avsum = bpool.tile(
                                [HD, QBLK], F32, tag="avsum", name="avsum"
                            )
                            nc.vector.tensor_add(
                                avsum[:], avs[h][:HD, :], avs2[h][:HD, :]
                            )
                            nc.vector.tensor_mul(dst, avsum[:], bc[:])
                if not (os.environ.get("V3_NO_AV") or os.environ.get("V3_NO_NORM")):
                    nc.gpsimd.dma_start(
                        attn_sb[HD:128, :, qb * QBLK : (qb + 1) * QBLK], tmp_all[:]
                    )

        # ---------------- phase 3: output projection (bf16) ----------------
        with ExitStack() as p3:
            ypool = p3.enter_context(tc.tile_pool(name="y", bufs=3))
            ps_y = p3.enter_context(tc.tile_pool(name="ps_y", bufs=4, space="PSUM"))
            y_r = y_d.rearrange("(st p) o -> st p o", p=128)
            for st in range(ST):
                y_sb = ypool.tile([128, C], F32, tag="ysb", name="y_sb")
                for nb2 in range(2):
                    ps = ps_y.tile([128, 384], F32, tag="psy", name="psy")
                    for k3 in range(3):
                        nc.tensor.matmul(
                            ps[:],
                            attn_sb[:, k3, st * 128 : (st + 1) * 128],
                            wproj_sb[:, k3, nb2 * 384 : (nb2 + 1) * 384],
                            start=(k3 == 0),
                            stop=(k3 == 2),
                        )
                    if nb2 == 0:
                        nc.vector.tensor_copy(y_sb[:, :384], ps[:])
                    else:
                        nc.scalar.copy(y_sb[:, 384:], ps[:])
                nc.sync.dma_start(y_r[st], y_sb[:])

    nc.compile()
    return nc


def _prep_core_inputs(x, mask, w_qkv, w_proj, core):
    import ml_dtypes

    f8 = ml_dtypes.float8_e4m3
    f8e5 = ml_dtypes.float8_e5m2
    bf16 = ml_dtypes.bfloat16

    b, g = core // 2, core % 2
    s0, s1 = D_CORE * g, D_CORE * (g + 1)

    def cmajor(a):  # [C, n] -> [128, 6, n]
        return np.ascontiguousarray(a.reshape(6, 128, a.shape[1]).transpose(1, 0, 2))

    xT = np.ascontiguousarray(x[b].T)  # [C, S]
    wq = w_qkv[s0:s1, :].T * (HD ** -0.5)
    wk = w_qkv[C + s0 : C + s1, :].T
    wv = w_qkv[2 * C + s0 : 2 * C + s1, :].T

    maskT = mask[b].T  # [S(key), S(q)]
    m2 = np.zeros((128, KC, 2, S), dtype=f8e5)
    m2[:, :, 0, :] = (
        np.array([0.0, MASK_NEG], dtype=np.float32)[maskT]
        .reshape(KC, 128, S)
        .transpose(1, 0, 2)
        .astype(f8e5)
    )
    keep = (
        np.array([1.0, 0.0], dtype=np.float32)[maskT]
        .reshape(KC, 128, S)
        .transpose(1, 0, 2)
        .astype(bf16)
    )

    id8 = np.zeros((128, 2, 128), dtype=f8)
    id8[:, 0, :] = np.eye(128, dtype=np.float32).astype(f8)

    wproj = np.ascontiguousarray(w_proj[:, s0:s1].T)  # [384, C]
    wproj2 = wproj.reshape(3, 128, C).transpose(1, 0, 2)

    return {
        "xb": cmajor(xT).astype(bf16),
        "wqb": cmajor(np.ascontiguousarray(wq)).astype(bf16),
        "wkb": cmajor(np.ascontiguousarray(wk)).astype(bf16),
        "wvb": cmajor(np.ascontiguousarray(wv)).astype(bf16),
        "maske5": m2,
        "keepb": np.ascontiguousarray(keep),
        "ident8": id8,
        "wproj": np.ascontiguousarray(wproj2).astype(bf16),
    }


_NC_CACHE = {}


def get_nc():
    if "nc" not in _NC_CACHE:
        _NC_CACHE["nc"] = _build_kernel()
    return _NC_CACHE["nc"]


def _build_runner(nc):
    """Reusable jitted shard_map callable over the 8 cores."""
    import jax
    from jax.experimental.shard_map import shard_map
    from jax.sharding import Mesh, PartitionSpec

    from concourse.bass2jax import (
        _bass_exec_p,
        install_neuronx_cc_hook,
        partition_id_tensor,
    )

    install_neuronx_cc_hook()
    partition_name = nc.partition_id_tensor.name if nc.partition_id_tensor else None
    in_names, out_names, out_avals, zero_outs = [], [], [], []
    for alloc in nc.m.functions[0].allocations:
        if not isinstance(alloc, mybir.MemoryLocationSet):
            continue
        name = alloc.memorylocations[0].name
        if alloc.kind == "ExternalInput":
            if name != partition_name:
                in_names.append(name)
        elif alloc.kind == "ExternalOutput":
            out_names.append(name)
            shape = tuple(alloc.tensor_shape)
            dtype = mybir.dt.np(alloc.dtype)
            out_avals.append(jax.core.ShapedArray(shape, dtype))
            zero_outs.append(np.zeros(shape, dtype))
    n_params = len(in_names)
    all_in_names = list(in_names) + list(out_names)
    if partition_name is not None:
        all_in_names.append(partition_name)

    def _body(*args):
        operands = list(args)
        if partition_name is not None:
            operands.append(partition_id_tensor())
        outs = _bass_exec_p.bind(
            *operands,
            out_avals=tuple(out_avals),
            in_names=tuple(all_in_names),
            out_names=tuple(out_names),
            lowering_input_output_aliases=(),
            sim_require_finite=True,
            sim_require_nnan=True,
            nc=nc,
        )
        return tuple(outs)

    n_cores = nc.num_devices
    devices = jax.devices()[:n_cores]
    mesh = Mesh(np.asarray(devices), ("core",))
    in_specs = (PartitionSpec("core"),) * (n_params + len(out_names))
    out_specs = (PartitionSpec("core"),) * len(out_names)
    fn = jax.jit(
        shard_map(
            _body, mesh=mesh, in_specs=in_specs, out_specs=out_specs, check_rep=False
        ),
        keep_unused=True,
    )
    return fn, in_names, out_names, zero_outs


_RUNNER_CACHE = {}


def get_runner(nc, in_maps):
    """Return (fn, dev_args) for repeated dispatch of `nc` with `in_maps`."""
    import jax
    from jax.sharding import Mesh, NamedSharding, PartitionSpec

    key = id(nc)
    if key not in _RUNNER_CACHE:
        _RUNNER_CACHE[key] = _build_runner(nc)
    fn, in_names, out_names, zero_outs = _RUNNER_CACHE[key]
    n_cores = nc.num_devices
    mesh = Mesh(np.asarray(jax.devices()[:n_cores]), ("core",))
    shard = NamedSharding(mesh, PartitionSpec("core"))
    concat_in = [
        np.concatenate([np.asarray(in_maps[c][n]) for c in range(n_cores)], axis=0)
        for n in in_names
    ]
    dev_in = [jax.device_put(a, shard) for a in concat_in]
    zkey = ("zeros", key)
    if zkey not in _RUNNER_CACHE:
        concat_zeros = [
            np.zeros((n_cores * z.shape[0], *z.shape[1:]), z.dtype) for z in zero_outs
        ]
        _RUNNER_CACHE[zkey] = [jax.device_put(a, shard) for a in concat_zeros]
    return fn, dev_in + _RUNNER_CACHE[zkey]


def run_cached(nc, in_maps):
    """Execute via the cached runner; returns per-core result dicts."""
    fn, dev_args = get_runner(nc, in_maps)
    out_arrs = fn(*dev_args)
    _, _, out_names, zero_outs = _RUNNER_CACHE[id(nc)]
    n_cores = nc.num_devices
    fetched = [
        np.asarray(a).reshape(n_cores, *zero_outs[i].shape)
        for i, a in enumerate(out_arrs)
    ]
    return [
        {name: fetched[i][c] for i, name in enumerate(out_names)}
        for c in range(n_cores)
    ]


def make_in_maps(x, mask, w_qkv, w_proj):
    return [_prep_core_inputs(x, mask, w_qkv, w_proj, c) for c in range(N_CORES)]


def combine(results, b_proj):
    outs = []
    for b in range(B):
        outs.append(results[2 * b]["y"] + results[2 * b + 1]["y"] + b_proj[None, :])
    return np.stack(outs).astype(np.float32)


def kernel(x, mask, w_qkv, w_proj, b_proj):
    x = np.asarray(x, dtype=np.float32)
    mask = np.asarray(mask)
    w_qkv = np.asarray(w_qkv, dtype=np.float32)
    w_proj = np.asarray(w_proj, dtype=np.float32)
    b_proj = np.asarray(b_proj, dtype=np.float32)

    nc = get_nc()
    in_maps = make_in_maps(x, mask, w_qkv, w_proj)
    try:
        results = run_cached(nc, in_maps)
    except Exception:
        results = run_bass_kernel_spmd(nc, in_maps, list(range(N_CORES))).results
    return combine(results, b_proj)
